# revision 3
# baseline (speedup 1.0000x reference)
"""Trainium2 kernel for nn_Decoder (moe_routing).

Reference computation:
    h = relu(latent @ W1 + b1)                  # [B, NL] @ [NL, H] -> [B, H]
    h = (h - bn_mean) * rsqrt(bn_var + eps) * bn_gamma + bn_beta
    w = weight_table[genes_oi]                  # [G, H, C]
    out = einsum("bh,ghc->bgc", h, w) + bias_table[genes_oi]

Sharding: genes_oi / gathered weight table split along the gene axis across
8 cores (625 genes each, padded to 640).  The small MLP and latent batch are
replicated on every core.  The per-gene weight gather is performed on the
host as part of input sharding; each core receives a dense, pre-transposed
weight slab laid out for full-bandwidth DMA and direct use as the matmul
moving operand.

Device kernel (per core):
    zT = W1.T @ latent.T          (PE, K=128, M=64 twice -> [128(h dup), 256(b)])
    hT = (relu(zT + b1)) * s + t  (DVE/ACT, per-partition scalars)
    for each 64-gene block:
        stream weights [128, 2048] (two 32-gene halves on partition halves)
        matmul pairs: lhsT = hT chunk [64, 128], rhs = weights [64, 512]
        (row-tiled: lower half on PE rows 0-63, upper half on rows 64-127)
        PSUM -> SBUF copy -> 2MB DMA per b-chunk to DRAM
"""

import os
import numpy as np

B, NL, H, C = 256, 128, 64, 64
G = 5000
NCORES = 8
GC = G // NCORES          # 625 genes per core
GB = 64                   # genes per block on device
NB = 10                   # blocks per core
GP = GB * NB              # 640 padded genes per core
BN_EPS = 1e-5

# fp32r: PE streams fp32 at 4x rate (low 12 mantissa bits dropped, ~2.4e-4
# relative quantization).  Toggle with KERNEL_F32R=0/1.
USE_F32R = os.environ.get("KERNEL_F32R", "1") == "1"

_NC_CACHE = None
_LAST_RESULTS = None      # BassKernelResults of the most recent run (for test.py)


def _round_fp32_to_fp32r(a: np.ndarray) -> np.ndarray:
    """Round fp32 to fp32r (zero low 12 mantissa bits, round-to-nearest-even)."""
    u = np.ascontiguousarray(a, dtype=np.float32).view(np.uint32)
    lsb = (u >> 12) & 1
    r = (u + 0x7FF + lsb) & 0xFFFFF000
    return r.view(np.float32)


def _build_nc():
    from concourse import bacc, tile, mybir

    f32 = mybir.dt.float32
    f32r = mybir.dt.float32r
    nc = bacc.Bacc(
        "TRN2",
        target_bir_lowering=False,
        debug=False,
        num_devices=NCORES,
        enable_partition_id=False,
    )

    latT_d = nc.dram_tensor("latT", [NL, B], f32, kind="ExternalInput")
    w1_d = nc.dram_tensor("w1", [NL, H], f32, kind="ExternalInput")
    vec_d = nc.dram_tensor("vec", [2 * H, 3], f32, kind="ExternalInput")
    wg_d = nc.dram_tensor("wg", [2 * H, NB * GB * C // 2], f32, kind="ExternalInput")
    out_d = nc.dram_tensor("out", [B, GP * C], f32, kind="ExternalOutput")

    def mm_cast(ap):
        return ap.bitcast(f32r) if USE_F32R else ap

    with tile.TileContext(nc) as tc:
        with (
            tc.tile_pool(name="const", bufs=1) as cpool,
            tc.tile_pool(name="wpool", bufs=3) as wpool,
            tc.tile_pool(name="opool", bufs=4) as opool,
            tc.tile_pool(name="mlp_ps", bufs=1, space="PSUM") as mlp_ps,
            tc.tile_pool(name="ps", bufs=3, space="PSUM") as pspool,
        ):
            latT = cpool.tile([NL, B], f32)
            w1 = cpool.tile([NL, H], f32)
            vec = cpool.tile([2 * H, 3], f32)
            nc.sync.dma_start(latT[:], latT_d.ap()[:])
            nc.sync.dma_start(w1[:], w1_d.ap()[:])
            nc.sync.dma_start(vec[:], vec_d.ap()[:])

            # MLP: zT = W1.T @ latT, written twice so both partition halves
            # hold the same [H, B] activations (feeds both PE row groups).
            zT = mlp_ps.tile([2 * H, B], f32)
            nc.tensor.matmul(zT[0:H, :], w1[:], latT[:], start=True, stop=True)
            nc.tensor.matmul(zT[H : 2 * H, :], w1[:], latT[:], start=True, stop=True)

            u = cpool.tile([2 * H, B], f32)
            hT = cpool.tile([2 * H, B], f32)
            nc.vector.tensor_scalar(
                out=u[:], in0=zT[:], scalar1=vec[:, 0:1], scalar2=None,
                op0=mybir.AluOpType.add,
            )
            nc.scalar.activation(u[:], u[:], mybir.ActivationFunctionType.Relu)
            nc.vector.tensor_scalar(
                out=hT[:], in0=u[:], scalar1=vec[:, 1:2], scalar2=vec[:, 2:3],
                op0=mybir.AluOpType.mult, op1=mybir.AluOpType.add,
            )

            HB = GB * C // 2   # 2048: free size of one block's weight slab
            for blk in range(NB):
                wg = wpool.tile([2 * H, HB], f32)
                nc.sync.dma_start(wg[:], wg_d.ap()[:, blk * HB : (blk + 1) * HB])
                for chunk in range(2):
                    lhs_lo = mm_cast(hT[0:H, chunk * 128 : (chunk + 1) * 128])
                    lhs_hi = mm_cast(hT[H : 2 * H, chunk * 128 : (chunk + 1) * 128])
                    ob = opool.tile([128, GB * C], f32)
                    for sub in range(4):
                        ps_a = pspool.tile([128, 512], f32)
                        ps_b = pspool.tile([128, 512], f32)
                        rhs_lo = mm_cast(wg[0:H, sub * 512 : (sub + 1) * 512])
                        rhs_hi = mm_cast(wg[H : 2 * H, sub * 512 : (sub + 1) * 512])
                        nc.tensor.matmul(ps_a[:], lhs_lo, rhs_lo, start=True, stop=True)
                        nc.tensor.matmul(ps_b[:], lhs_hi, rhs_hi, start=True, stop=True)
                        nc.vector.tensor_copy(
                            ob[:, sub * 512 : (sub + 1) * 512], ps_a[:]
                        )
                        nc.vector.tensor_copy(
                            ob[:, 2048 + sub * 512 : 2048 + (sub + 1) * 512], ps_b[:]
                        )
                    nc.sync.dma_start(
                        out_d.ap()[
                            chunk * 128 : (chunk + 1) * 128,
                            blk * GB * C : (blk + 1) * GB * C,
                        ],
                        ob[:],
                    )

    nc.compile()
    return nc


def _get_nc():
    global _NC_CACHE
    if _NC_CACHE is None:
        _NC_CACHE = _build_nc()
    return _NC_CACHE


def _prepare_in_maps(latent, W1, b1, bn_gamma, bn_beta, bn_mean, bn_var,
                     weight_table, gid):
    s = bn_gamma / np.sqrt(bn_var + BN_EPS)
    t = bn_beta - bn_mean * s
    vec = np.stack([b1, s, t], axis=1).astype(np.float32)        # [64, 3]
    vec128 = np.ascontiguousarray(np.concatenate([vec, vec], 0))  # [128, 3]
    latT = np.ascontiguousarray(latent.T)                         # [128, 256]

    in_maps = []
    for c in range(NCORES):
        g = gid[c * GC : (c + 1) * GC]
        gp = np.concatenate([g, np.zeros(GP - GC, dtype=np.int64)])
        wt = weight_table[gp]                                     # [640, 64, 64]
        # [blk, half, j, h, c] -> [half, h, blk, j, c] -> [128, NB*2048]
        wdev = np.ascontiguousarray(
            wt.reshape(NB, 2, GB // 2, H, C)
            .transpose(1, 3, 0, 2, 4)
            .reshape(2 * H, NB * (GB // 2) * C)
        )
        if USE_F32R:
            wdev = _round_fp32_to_fp32r(wdev)
        in_maps.append({"latT": latT, "w1": W1, "vec": vec128, "wg": wdev})
    return in_maps


def _postprocess(results, gid, bias_table):
    outs = [results[c]["out"].reshape(B, GP, C)[:, :GC, :] for c in range(NCORES)]
    out = np.concatenate(outs, axis=1)
    bias_g = bias_table[gid]                                      # [G, C]
    if np.any(bias_g):
        out = out + bias_g[None, :, :]
    return np.ascontiguousarray(out)


def kernel(latent, genes_oi, W1, b1, bn_gamma, bn_beta, bn_mean, bn_var,
           weight_table, bias_table):
    global _LAST_RESULTS
    from concourse import bass_utils

    latent = np.asarray(latent, dtype=np.float32)
    W1 = np.ascontiguousarray(np.asarray(W1, dtype=np.float32))
    b1 = np.asarray(b1, dtype=np.float32)
    bn_gamma = np.asarray(bn_gamma, dtype=np.float32)
    bn_beta = np.asarray(bn_beta, dtype=np.float32)
    bn_mean = np.asarray(bn_mean, dtype=np.float32)
    bn_var = np.asarray(bn_var, dtype=np.float32)
    weight_table = np.asarray(weight_table, dtype=np.float32)
    bias_table = np.asarray(bias_table, dtype=np.float32)
    gid = np.asarray(genes_oi).astype(np.int64)

    in_maps = _prepare_in_maps(latent, W1, b1, bn_gamma, bn_beta, bn_mean,
                               bn_var, weight_table, gid)
    nc = _get_nc()
    res = bass_utils.run_bass_kernel_spmd(
        nc, in_maps, core_ids=list(range(NCORES)), trace=False
    )
    _LAST_RESULTS = res
    return _postprocess(res.results, gid, bias_table)


# revision 20
# speedup vs baseline: 9.9741x; 9.9741x over previous
"""Trainium2 kernel for nn_Decoder (moe_routing).

Reference computation:
    h = relu(latent @ W1 + b1)                  # [B, NL] @ [NL, H] -> [B, H]
    h = (h - bn_mean) * rsqrt(bn_var + eps) * bn_gamma + bn_beta
    w = weight_table[genes_oi]                  # [G, H, C]
    out = einsum("bh,ghc->bgc", h, w) + bias_table[genes_oi]

Sharding: genes_oi / gathered weight table split along the gene axis across
8 cores (625 genes each, padded to 640).  The small MLP and latent batch are
replicated on every core.  The per-gene weight gather is performed on the
host as part of input sharding; each core receives a dense, pre-transposed
weight slab laid out for full-bandwidth DMA and direct use as the matmul
moving operand.

Device kernel (per core):
    zT = W1.T @ latent.T          (PE, K=128, M=64 twice -> [128(h dup), 256(b)])
    hT = (relu(zT + b1)) * s + t  (DVE/ACT, per-partition scalars)
    for each 64-gene block:
        stream weights [128, 2048] (two 32-gene halves on partition halves)
        matmul pairs: lhsT = hT chunk [64, 128], rhs = weights [64, 512]
        (row-tiled: lower half on PE rows 0-63, upper half on rows 64-127)
        PSUM -> SBUF copy -> 2MB DMA per b-chunk to DRAM
"""

import os
import numpy as np

B, NL, H, C = 256, 128, 64, 64
G = 5000
NCORES = 8
GC = G // NCORES          # 625 genes per core
GB = 64                   # genes per block on device
NB = 10                   # blocks per core
GP = GB * NB              # 640 padded genes per core
BN_EPS = 1e-5

# Matmul operand dtype for the gene einsum:
#   f32r (default): PE streams fp32 at 4x rate (low 12 mantissa bits dropped,
#                   ~2.4e-4 relative quantization)
#   f32:            exact, but 4 cycles/row on the PE
#   bf16:           halves the weight-stream DMA, ~2e-3 relative error
if os.environ.get("KERNEL_F32R") == "0":
    _WDT_DEFAULT = "f32"
else:
    _WDT_DEFAULT = "f32r"
WDT = os.environ.get("KERNEL_WDT", _WDT_DEFAULT)
USE_F32R = WDT == "f32r"

_NC_CACHE = None
_LAST_RESULTS = None      # BassKernelResults of the most recent run (for test.py)


def _round_fp32_to_fp32r(a: np.ndarray) -> np.ndarray:
    """Round fp32 to fp32r (zero low 12 mantissa bits, round-to-nearest-even)."""
    u = np.ascontiguousarray(a, dtype=np.float32).view(np.uint32)
    lsb = (u >> 12) & 1
    r = (u + 0x7FF + lsb) & 0xFFFFF000
    return r.view(np.float32)


def _build_nc(loop_r=None, wbufs=4, obufs=6, psbufs=3, act_copy_every=2,
              paired_psum=True, wsuper=1, osplit=1, wdma_act=False):
    """Build the Bass program.  loop_r: if set, repeat the whole pipeline
    loop_r times on device inside a hardware loop (benchmarking only —
    output is rewritten identically every iteration)."""
    from concourse import bacc, tile, mybir

    f32 = mybir.dt.float32
    f32r = mybir.dt.float32r
    bf16 = mybir.dt.bfloat16
    nc = bacc.Bacc(
        "TRN2",
        target_bir_lowering=False,
        debug=False,
        num_devices=NCORES,
        enable_partition_id=False,
    )

    fmm = {"f32r": f32r, "bf16": bf16, "f32": f32}[WDT]
    latT_d = nc.dram_tensor("latT", [NL, B], f32, kind="ExternalInput")
    w1_d = nc.dram_tensor("w1", [NL, H], f32, kind="ExternalInput")
    vec_d = nc.dram_tensor("vec", [2 * H, 3], f32, kind="ExternalInput")
    wg_d = nc.dram_tensor("wg", [2 * H, NB * GB * C // 2], fmm, kind="ExternalInput")
    out_d = nc.dram_tensor("out", [B, GP * C], f32, kind="ExternalOutput")

    with tile.TileContext(nc) as tc:
        with (
            tc.tile_pool(name="const", bufs=1) as cpool,
            tc.tile_pool(name="wpool", bufs=wbufs) as wpool,
            tc.tile_pool(name="opool", bufs=obufs) as opool,
            tc.tile_pool(name="mlp_ps", bufs=1, space="PSUM") as mlp_ps,
            tc.tile_pool(name="ps", bufs=psbufs, space="PSUM") as pspool,
        ):
          import contextlib
          loop_cm = tc.For_i(0, loop_r, 1) if loop_r else contextlib.nullcontext()
          with loop_cm:
            latT = cpool.tile([NL, B], f32)
            w1 = cpool.tile([NL, H], f32)
            vec = cpool.tile([2 * H, 3], f32)
            nc.sync.dma_start(latT[:], latT_d.ap()[:])
            nc.sync.dma_start(w1[:], w1_d.ap()[:])
            nc.sync.dma_start(vec[:], vec_d.ap()[:])

            # MLP: zT = W1.T @ latT, written twice so both partition halves
            # hold the same [H, B] activations (feeds both PE row groups).
            zT = mlp_ps.tile([2 * H, B], f32)
            nc.tensor.matmul(zT[0:H, :], w1[:], latT[:], start=True, stop=True)
            nc.tensor.matmul(zT[H : 2 * H, :], w1[:], latT[:], start=True, stop=True)

            u = cpool.tile([2 * H, B], f32)
            hT = cpool.tile([2 * H, B], fmm)
            nc.vector.tensor_scalar(
                out=u[:], in0=zT[:], scalar1=vec[:, 0:1], scalar2=None,
                op0=mybir.AluOpType.add,
            )
            nc.scalar.activation(u[:], u[:], mybir.ActivationFunctionType.Relu)
            nc.vector.tensor_scalar(
                out=hT[:], in0=u[:], scalar1=vec[:, 1:2], scalar2=vec[:, 2:3],
                op0=mybir.AluOpType.mult, op1=mybir.AluOpType.add,
            )

            HB = GB * C // 2   # 2048: free size of one block's weight slab
            wg_super = None
            for blk in range(NB):
                if blk % wsuper == 0:
                    nsup = min(wsuper, NB - blk)
                    wg_super = wpool.tile([2 * H, HB * nsup], fmm)
                    wdma = nc.scalar if wdma_act else nc.sync
                    wdma.dma_start(
                        wg_super[:],
                        wg_d.ap()[:, blk * HB : (blk + nsup) * HB],
                    )
                wg = wg_super[:, (blk % wsuper) * HB : (blk % wsuper + 1) * HB]
                for chunk in range(2):
                    lhs_lo = hT[0:H, chunk * 128 : (chunk + 1) * 128]
                    lhs_hi = hT[H : 2 * H, chunk * 128 : (chunk + 1) * 128]
                    ob = opool.tile([128, GB * C], f32)
                    copy_i = 0

                    def cp(dst, src):
                        nonlocal copy_i
                        copy_i += 1
                        if act_copy_every and copy_i % act_copy_every == 0:
                            nc.scalar.copy(dst, src)
                        else:
                            nc.vector.tensor_copy(dst, src)

                    # sub s covers block-genes [16s, 16s+16): the first 8 on
                    # partitions 0-63 (PE rows 0-63), the next 8 on 64-127.
                    # The PSUM pair is therefore contiguous in the out tile.
                    for sub in range(4):
                        rhs_lo = wg[0:H, sub * 512 : (sub + 1) * 512]
                        rhs_hi = wg[H : 2 * H, sub * 512 : (sub + 1) * 512]
                        if paired_psum:
                            ps = pspool.tile([128, 1024], f32)
                            ps_a = ps[:, 0:512]
                            ps_b = ps[:, 512:1024]
                        else:
                            ps_a = pspool.tile([128, 512], f32)
                            ps_b = pspool.tile([128, 512], f32)
                        nc.tensor.matmul(ps_a, lhs_lo, rhs_lo, start=True, stop=True)
                        nc.tensor.matmul(ps_b, lhs_hi, rhs_hi, start=True, stop=True)
                        if paired_psum:
                            cp(ob[:, sub * 1024 : (sub + 1) * 1024], ps[:])
                        else:
                            cp(ob[:, sub * 1024 : sub * 1024 + 512], ps_a)
                            cp(ob[:, sub * 1024 + 512 : (sub + 1) * 1024], ps_b)
                        done = (sub + 1) * 1024
                        piece = GB * C // osplit
                        if done % piece == 0:
                            lo = done - piece
                            nc.sync.dma_start(
                                out_d.ap()[
                                    chunk * 128 : (chunk + 1) * 128,
                                    blk * GB * C + lo : blk * GB * C + done,
                                ],
                                ob[:, lo:done],
                            )

    nc.compile()
    return nc


def _get_nc():
    global _NC_CACHE
    if _NC_CACHE is None:
        _NC_CACHE = _build_nc()
    return _NC_CACHE


def _prepare_in_maps(latent, W1, b1, bn_gamma, bn_beta, bn_mean, bn_var,
                     weight_table, gid):
    s = bn_gamma / np.sqrt(bn_var + BN_EPS)
    t = bn_beta - bn_mean * s
    vec = np.stack([b1, s, t], axis=1).astype(np.float32)        # [64, 3]
    vec128 = np.ascontiguousarray(np.concatenate([vec, vec], 0))  # [128, 3]
    latT = np.ascontiguousarray(latent.T)                         # [128, 256]

    in_maps = []
    for c in range(NCORES):
        g = gid[c * GC : (c + 1) * GC]
        gp = np.concatenate([g, np.zeros(GP - GC, dtype=np.int64)])
        wt = weight_table[gp]                                     # [640, 64, 64]
        # [blk, sub, half, j, h, c] -> [half, h, blk, sub, j, c] -> [128, NB*2048]
        wdev = np.ascontiguousarray(
            wt.reshape(NB, 4, 2, 8, H, C)
            .transpose(2, 4, 0, 1, 3, 5)
            .reshape(2 * H, NB * (GB // 2) * C)
        )
        if WDT == "f32r":
            wdev = _round_fp32_to_fp32r(wdev)
        elif WDT == "bf16":
            import ml_dtypes
            wdev = wdev.astype(ml_dtypes.bfloat16)
        in_maps.append({"latT": latT, "w1": W1, "vec": vec128, "wg": wdev})
    return in_maps


def _postprocess(results, gid, bias_table):
    outs = [results[c]["out"].reshape(B, GP, C)[:, :GC, :] for c in range(NCORES)]
    out = np.concatenate(outs, axis=1)
    bias_g = bias_table[gid]                                      # [G, C]
    if np.any(bias_g):
        out = out + bias_g[None, :, :]
    return np.ascontiguousarray(out)


def kernel(latent, genes_oi, W1, b1, bn_gamma, bn_beta, bn_mean, bn_var,
           weight_table, bias_table):
    global _LAST_RESULTS
    from concourse import bass_utils

    latent = np.asarray(latent, dtype=np.float32)
    W1 = np.ascontiguousarray(np.asarray(W1, dtype=np.float32))
    b1 = np.asarray(b1, dtype=np.float32)
    bn_gamma = np.asarray(bn_gamma, dtype=np.float32)
    bn_beta = np.asarray(bn_beta, dtype=np.float32)
    bn_mean = np.asarray(bn_mean, dtype=np.float32)
    bn_var = np.asarray(bn_var, dtype=np.float32)
    weight_table = np.asarray(weight_table, dtype=np.float32)
    bias_table = np.asarray(bias_table, dtype=np.float32)
    gid = np.asarray(genes_oi).astype(np.int64)

    in_maps = _prepare_in_maps(latent, W1, b1, bn_gamma, bn_beta, bn_mean,
                               bn_var, weight_table, gid)
    nc = _get_nc()
    res = bass_utils.run_bass_kernel_spmd(
        nc, in_maps, core_ids=list(range(NCORES)), trace=False
    )
    _LAST_RESULTS = res
    return _postprocess(res.results, gid, bias_table)
